# revision 17
# baseline (speedup 1.0000x reference)
"""Trainium2 Bass kernel for a pre-LN transformer block (B=4, S=2048, H=12, D=64).

Sharding: 8 cores; core c -> batch b = c//2, parity p = c%2.
Each core handles 1024 query rows of its batch: local query block j (128 rows,
j=0..7) maps to global block g = 2j + p (stride-2 interleave balances the causal
load so every core runs an identical SPMD program).

Per core (activations feature-major [feature, token]):
  LN1 (stats via ones-matmul over partitions, a/c bounced through DRAM for a
  [128,n] reshape, then broadcast), QKV projections, causal attention
  (scores transposed [keys, q], key-tile-outer; head pairs run concurrently on
  PE row-groups; softmax without max-subtraction; multiplicative masks from
  host; normalization decoupled from the accumulation critical path),
  Wo + residual, LN2 (fp32 stats), MLP (exact GELU, token-half sweeps, hidden
  activations stay in SBUF) + residual.

LayerNorm gains are folded into the consumer weights on the host
(W <- diag(ln_w) @ W), so the on-device LN is just u = x*rstd - mu*rstd.
When all bias vectors are zero (they are for this model), PSUM evacuations run
on the Scalar engine; otherwise a Vector-engine bias path is emitted.
"""

import numpy as np

N_CORES = 8
B, S, H, D = 4, 2048, 12, 64
HID = 768
QL = 1024
KT = HID // 128
TT = S // 128
MH = 4 * HID // 128
EPS = 1e-5

_CACHE = {}


def _build_program(biases_zero, debug=False):
    from contextlib import ExitStack
    import concourse.bass as bass
    import concourse.tile as tile
    from concourse import bacc, mybir

    F32 = mybir.dt.float32
    BF16 = mybir.dt.bfloat16
    Alu = mybir.AluOpType
    Act = mybir.ActivationFunctionType

    nc = bacc.Bacc("TRN2", target_bir_lowering=False, debug=False,
                   enable_asserts=False, num_devices=N_CORES)

    def din(name, shape, dt):
        return nc.dram_tensor(name, shape, dt, kind="ExternalInput").ap()

    xbT = din("xbT", [HID, S], BF16)
    xbTq = din("xbTq", [HID, QL], BF16)
    xTq = din("xTq", [HID, QL], F32)
    masks = din("masks", [TT, 128, 128], BF16)
    Wq = din("Wq", [HID, HID], BF16)      # pre-folded with ln1_w on host
    Wk = din("Wk", [HID, HID], BF16)
    Wv = din("Wv", [HID, HID], BF16)
    Wo = din("Wo", [HID, HID], BF16)
    W1 = din("W1", [HID, 4 * HID], BF16)  # pre-folded with ln2_w
    W2 = din("W2", [4 * HID, HID], BF16)
    bqs = din("bqs", [HID], F32)          # (Wq^T ln1_b + bq)/sqrt(D)
    bk = din("bk", [HID], F32)            # Wk^T ln1_b + bk
    bv = din("bv", [HID], F32)
    bo = din("bo", [HID], F32)
    b1 = din("b1", [4 * HID], F32)        # W1^T ln2_b + b1
    b2 = din("b2", [HID], F32)

    y = nc.dram_tensor("y", [HID, QL], F32, kind="ExternalOutput").ap()
    dbg = {}
    if debug:
        for nm, shp in [("ln_d", [HID, S]), ("lnq_d", [HID, QL]),
                        ("K_d", [HID, S]), ("Q_d", [HID, QL]),
                        ("V_d", [TT * 128, H * 65]), ("attn_d", [HID, QL]),
                        ("r_d", [HID, QL]), ("ln2_d", [HID, QL])]:
            dbg[nm] = nc.dram_tensor(nm, shp, F32, kind="ExternalOutput").ap()

    st_dram = nc.dram_tensor("st_scratch", [16, S], F32).ap()
    rec_dram = nc.dram_tensor("rec_scratch", [H, QL], F32).ap()
    rec2_dram = nc.dram_tensor("rec2_scratch", [H, QL], F32).ap()

    def bcast(src_elem_ap, parts, n):
        return bass.AP(tensor=src_elem_ap.tensor, offset=src_elem_ap.offset,
                       ap=[[0, parts], [1, n]])

    def resh_ap(dram, row_elem_ap, nb):
        return bass.AP(tensor=dram.tensor, offset=row_elem_ap.offset,
                       ap=[[1, 128], [128, nb]])

    with tile.TileContext(nc) as tc, ExitStack() as ctx:
        sb = ctx.enter_context(tc.tile_pool(name="sb", bufs=1))
        ps = ctx.enter_context(tc.tile_pool(name="ps", bufs=1, space="PSUM"))

        def pst_tile(name, shape=(128, 1024), dt=F32):
            return ps.tile(list(shape), dt, tag="s2", bufs=4, name=name,
                           padded_shape=[128, 1024])

        # ---------- constants ----------
        ones_bf = sb.tile([128, 1], BF16, tag="ones")
        nc.vector.memset(ones_bf, 1.0)
        ones_f = sb.tile([128, 1], F32, tag="onesf")
        nc.vector.memset(ones_f, 1.0)
        par = sb.tile([128, 80], F32, tag="par")

        def load_cols(dst0, src, n):
            nc.gpsimd.dma_start(
                out=par[:, dst0:dst0 + n],
                in_=bass.AP(tensor=src.tensor, offset=src.offset,
                            ap=[[1, 128], [128, n]]))

        # cols: 24:30 bqs, 30:36 bk, 36:42 bo, 42:48 b2, 48:72 b1, 72 eps
        load_cols(24, bqs, KT)
        load_cols(30, bk, KT)
        load_cols(36, bo, KT)
        load_cols(42, b2, KT)
        load_cols(48, b1, MH)
        nc.vector.memset(par[:, 72:73], EPS)
        eps_t = par[:, 72:73]
        bv_b = sb.tile([128, HID], F32, tag="bv_b")
        nc.gpsimd.dma_start(out=bv_b, in_=bcast(bv[0], 128, HID))

        # PE warmup: dummy matmuls with no data deps lift the HAM clock gate
        warm = sb.tile([128, 512], BF16, tag="sq", bufs=2, name="warm")
        nc.vector.memset(warm, 0.0)
        wps = pst_tile("warmps", (1, 512))
        for i in range(20):
            nc.tensor.matmul(wps, ones_bf, warm, start=True, stop=True)

        masks_sb = sb.tile([128, TT, 128], BF16, tag="masks")
        nc.gpsimd.dma_start(out=masks_sb, in_=masks.rearrange("t p q -> p t q"))

        # ---------- LN helper: out = x*rstd - mu*rstd (gains folded into W) ----
        def emit_ln(N, x_bf, out_bf, row0, f32_stats=False):
            nchunks = N // 512
            one = ones_f if f32_stats else ones_bf
            for c in range(nchunks):
                off = 512 * c
                cs = slice(off, off + 512)
                s_ps = pst_tile(f"sps{row0}_{c}", (1, 512))
                q_ps = pst_tile(f"qps{row0}_{c}", (1, 512))
                for kt in range(KT):
                    nc.tensor.matmul(s_ps, one, x_bf[:, kt, cs],
                                     start=(kt == 0), stop=(kt == KT - 1))
                for kt in range(KT):
                    sqd = F32 if f32_stats else BF16
                    sqc = sb.tile([128, 512], sqd, tag="sq", bufs=2,
                                  name=f"sq{row0}_{c}_{kt}")
                    nc.vector.tensor_mul(sqc, x_bf[:, kt, cs], x_bf[:, kt, cs])
                    nc.tensor.matmul(q_ps, one, sqc,
                                     start=(kt == 0), stop=(kt == KT - 1))
                s_sb = sb.tile([1, 512], F32, tag="sts", bufs=2, name=f"ssb{row0}_{c}")
                q_sb = sb.tile([1, 512], F32, tag="sts", bufs=2, name=f"qsb{row0}_{c}")
                nc.scalar.copy(s_sb, s_ps)
                nc.scalar.copy(q_sb, q_ps)
                nc.scalar.dma_start(out=st_dram[row0, off:off + 512], in_=s_sb)
                nc.scalar.dma_start(out=st_dram[row0 + 1, off:off + 512], in_=q_sb)
                nc.tensor.matmul(wps[:, 0:128], ones_f[0:1, :], s_sb[:, 0:128],
                                 start=True, stop=True)
                s_r = sb.tile([128, 4], F32, tag="str", bufs=8, name=f"sr{row0}_{c}")
                q_r = sb.tile([128, 4], F32, tag="str", bufs=8, name=f"qr{row0}_{c}")
                nc.scalar.dma_start(out=s_r, in_=resh_ap(st_dram, st_dram[row0, off], 4))
                nc.scalar.dma_start(out=q_r, in_=resh_ap(st_dram, st_dram[row0 + 1, off], 4))
                nc.tensor.matmul(wps[:, 0:4], ones_f, s_r, start=True, stop=True)
                mu = sb.tile([128, 4], F32, tag="str", bufs=8, name=f"mu{row0}_{c}")
                e2 = sb.tile([128, 4], F32, tag="str", bufs=8, name=f"e2{row0}_{c}")
                nc.scalar.mul(mu, s_r, 1.0 / HID)
                nc.scalar.mul(e2, q_r, 1.0 / HID)
                var = sb.tile([128, 4], F32, tag="str", bufs=8, name=f"var{row0}_{c}")
                nc.vector.tensor_mul(var, mu, mu)
                nc.vector.tensor_sub(var, e2, var)
                sd = sb.tile([128, 4], F32, tag="str", bufs=8, name=f"sd{row0}_{c}")
                nc.scalar.activation(sd, var, Act.Sqrt, bias=eps_t, scale=1.0)
                a_r = sb.tile([128, 4], F32, tag="str", bufs=8, name=f"ar{row0}_{c}")
                nc.vector.reciprocal(a_r, sd)
                c_r = sb.tile([128, 4], F32, tag="str", bufs=8, name=f"cr{row0}_{c}")
                nc.vector.tensor_mul(c_r, mu, a_r)
                nc.scalar.mul(c_r, c_r, -1.0)
                nc.scalar.dma_start(out=resh_ap(st_dram, st_dram[row0 + 2, off], 4), in_=a_r)
                nc.scalar.dma_start(out=resh_ap(st_dram, st_dram[row0 + 3, off], 4), in_=c_r)
                nc.tensor.matmul(wps[:, 0:4], ones_f, c_r, start=True, stop=True)
                a_b = sb.tile([128, 512], F32, tag="ab", bufs=2, name=f"ab{row0}_{c}")
                c_b = sb.tile([128, 512], F32, tag="ab", bufs=2, name=f"cb{row0}_{c}")
                nc.scalar.dma_start(out=a_b, in_=bcast(st_dram[row0 + 2, off], 128, 512))
                nc.scalar.dma_start(out=c_b, in_=bcast(st_dram[row0 + 3, off], 128, 512))
                nc.tensor.matmul(wps[:, 0:128], ones_f, a_b[:, 0:128],
                                 start=True, stop=True)
                nc.tensor.matmul(wps[:, 0:128], ones_f, c_b[:, 0:128],
                                 start=True, stop=True)
                for kt in range(KT):
                    t0 = sb.tile([128, 512], F32, tag="t0", bufs=2,
                                 name=f"t0{row0}_{c}_{kt}")
                    nc.vector.tensor_mul(t0, x_bf[:, kt, cs], a_b)
                    nc.vector.tensor_add(out_bf[:, kt, cs], t0, c_b)

        # evacuation helper: psum -> sbuf bf16 (+ optional bias col, scale)
        def evac(dst, src, bias_col=None, scale=1.0):
            if biases_zero or bias_col is None:
                if scale == 1.0:
                    nc.scalar.copy(dst, src)
                else:
                    nc.scalar.mul(dst, src, scale)
            else:
                nc.vector.tensor_scalar(dst, src, scale, bias_col,
                                        Alu.mult, Alu.add)

        # ---------- Phase 1: LN1 + LN1q ----------
        xbT_sb = sb.tile([128, KT, S], BF16, tag="fatA")
        for c in range(S // 512):
            nc.sync.dma_start(
                out=xbT_sb[:, :, 512 * c:512 * c + 512],
                in_=xbT.rearrange("(k p) s -> p k s", p=128)[:, :, 512 * c:512 * c + 512])
        xbTq_sb = sb.tile([128, KT, QL], BF16, tag="medB")
        for c in range(QL // 512):
            nc.sync.dma_start(
                out=xbTq_sb[:, :, 512 * c:512 * c + 512],
                in_=xbTq.rearrange("(k p) s -> p k s", p=128)[:, :, 512 * c:512 * c + 512])

        ln_bf = sb.tile([128, KT, S], BF16, tag="fatB")
        lnq_bf = sb.tile([128, KT, QL], BF16, tag="medA")
        emit_ln(S, xbT_sb, ln_bf, 0)
        emit_ln(QL, xbTq_sb, lnq_bf, 4)

        # ---------- Phase 2: QKV ----------
        K_sb = sb.tile([128, KT, S], BF16, tag="fatC")
        for n in range(S // 512):
            cs = slice(512 * n, 512 * n + 512)
            for mo in range(KT):
                wkt = sb.tile([128, KT, 128], BF16, tag="wk6", bufs=3,
                              name=f"wk{n}_{mo}")
                nc.sync.dma_start(
                    out=wkt,
                    in_=Wk.rearrange("(k p) m -> p k m", p=128)[:, :, 128 * mo:128 * mo + 128])
                pst = pst_tile(f"kps{mo}_{n}", (128, 512))
                for kt in range(KT):
                    nc.tensor.matmul(pst, wkt[:, kt, :], ln_bf[:, kt, cs],
                                     start=(kt == 0), stop=(kt == KT - 1))
                evac(K_sb[:, mo, cs], pst, par[:, 30 + mo:31 + mo])

        Q_sb = sb.tile([128, KT, QL], BF16, tag="qsb")
        for n in range(QL // 512):
            cs = slice(512 * n, 512 * n + 512)
            for mo in range(KT):
                wqt = sb.tile([128, KT, 128], BF16, tag="wk6", bufs=3,
                              name=f"wq{n}_{mo}")
                nc.sync.dma_start(
                    out=wqt,
                    in_=Wq.rearrange("(k p) m -> p k m", p=128)[:, :, 128 * mo:128 * mo + 128])
                pst = pst_tile(f"qps2{mo}_{n}", (128, 512))
                for kt in range(KT):
                    nc.tensor.matmul(pst, wqt[:, kt, :], lnq_bf[:, kt, cs],
                                     start=(kt == 0), stop=(kt == KT - 1))
                evac(Q_sb[:, mo, cs], pst, par[:, 24 + mo:25 + mo], scale=0.125)

        V_sb = sb.tile([128, TT, H * 65], BF16, tag="vsb")
        for fc in range(2):
            wvt = sb.tile([128, KT, 384], BF16, tag="wv", bufs=1, name=f"wv{fc}")
            nc.sync.dma_start(
                out=wvt,
                in_=Wv.rearrange("(k p) m -> p k m", p=128)[:, :, 384 * fc:384 * fc + 384])
            for tt in range(TT):
                pst = pst_tile(f"vps{tt}_{fc}", (128, 384))
                for kt in range(KT):
                    nc.tensor.matmul(pst, ln_bf[:, kt, 128 * tt:128 * tt + 128],
                                     wvt[:, kt, :],
                                     start=(kt == 0), stop=(kt == KT - 1))
                vdst = V_sb[:, tt, :].rearrange("p (h e) -> p h e", e=65)[:, 6 * fc:6 * fc + 6, 0:64]
                if biases_zero:
                    nc.scalar.copy(vdst, pst.rearrange("p (h d) -> p h d", d=64))
                else:
                    nc.vector.tensor_tensor(
                        vdst, pst.rearrange("p (h d) -> p h d", d=64),
                        bv_b[:, 384 * fc:384 * fc + 384].rearrange("p (h d) -> p h d", d=64),
                        Alu.add)
        for tt in range(TT):
            nc.vector.memset(
                V_sb[:, tt, :].rearrange("p (h e) -> p h e", e=65)[:, :, 64:65], 1.0)

        # ---------- Phase 3: attention (head pairs on PE row groups) ----------
        attn_bf = sb.tile([128, KT, QL], BF16, tag="medA")
        for kt in range(KT):
            h0, h1 = 2 * kt, 2 * kt + 1
            O = {h0: pst_tile(f"o{h0}", (65, QL)),
                 h1: pst_tile(f"o{h1}", (65, QL))}

            def av(h, tile_t, e, q0):
                for (cs, ce) in ([(q0, 512), (512, QL)] if q0 < 512 else [(q0, QL)]):
                    nc.tensor.matmul(O[h][:, cs:ce],
                                     V_sb[:, tile_t, 65 * h:65 * h + 65],
                                     e[:, cs - q0:ce - q0],
                                     start=(tile_t == 0), stop=(tile_t == TT - 1))

            prev = []
            for t in range(TT):
                q0 = 128 * (t // 2)
                span = QL - q0
                cur = []
                for h, pr in ((h0, slice(0, 64)), (h1, slice(64, 128))):
                    S_ps = pst_tile(f"scr{h}_{t}", (128, QL))
                    for (cs, ce) in ([(q0, 512), (512, QL)] if q0 < 512
                                     else [(q0, QL)]):
                        nc.tensor.matmul(S_ps[:, cs:ce],
                                         K_sb[pr, kt, 128 * t:128 * t + 128],
                                         Q_sb[pr, kt, cs:ce], start=True, stop=True)
                    expS = sb.tile([128, span], BF16, tag="expS", bufs=4,
                                   name=f"es{h}_{t}")
                    nc.scalar.activation(expS, S_ps[:, q0:QL], Act.Exp)
                    nc.vector.tensor_mul(expS[:, 0:128], expS[:, 0:128],
                                         masks_sb[:, t, :])
                    cur.append((h, expS, q0, t))
                for (h, e, pq0, pt) in prev:
                    av(h, pt, e, pq0)
                prev = cur
            for (h, e, pq0, pt) in prev:
                av(h, pt, e, pq0)
            # fast raw evacuation (frees O); normalization decoupled below
            for h in (h0, h1):
                rec = sb.tile([65, QL], F32, tag="rec", bufs=2, name=f"rec{h}")
                nc.vector.tensor_copy(rec[64:65, :], O[h][64:65, :])
                if h % 2 == 0:
                    nc.vector.tensor_copy(attn_bf[0:64, kt, :], O[h][0:64, :])
                else:
                    stg = sb.tile([64, QL], BF16, tag="stg", bufs=1, name=f"stg{h}")
                    nc.vector.tensor_copy(stg, O[h][0:64, :])
                nc.scalar.dma_start(out=rec_dram[h, :], in_=rec[64:65, :])
                rr = sb.tile([128, 8], F32, tag="str", bufs=8, name=f"rr{h}")
                nc.scalar.dma_start(out=rr, in_=resh_ap(rec_dram, rec_dram[h, 0], 8))
                rr2 = sb.tile([128, 8], F32, tag="str", bufs=8, name=f"rr2{h}")
                nc.vector.reciprocal(rr2, rr)
                nc.scalar.dma_start(out=resh_ap(rec2_dram, rec2_dram[h, 0], 8), in_=rr2)
                nc.scalar.dma_start(out=rec[0:64, :], in_=bcast(rec2_dram[h, 0], 64, QL))
                if h % 2 == 0:
                    nc.vector.tensor_mul(attn_bf[0:64, kt, :],
                                         attn_bf[0:64, kt, :], rec[0:64, :])
                else:
                    nc.vector.tensor_mul(stg, stg, rec[0:64, :])
                    nc.sync.dma_start(out=attn_bf[64:128, kt, :], in_=stg)

        if debug:
            for kt in range(KT):
                rs = slice(128 * kt, 128 * kt + 128)
                nc.gpsimd.dma_start(out=dbg["ln_d"][rs, :], in_=ln_bf[:, kt, :])
                nc.gpsimd.dma_start(out=dbg["lnq_d"][rs, :], in_=lnq_bf[:, kt, :])
                nc.gpsimd.dma_start(out=dbg["K_d"][rs, :], in_=K_sb[:, kt, :])
                nc.gpsimd.dma_start(out=dbg["Q_d"][rs, :], in_=Q_sb[:, kt, :])
                nc.gpsimd.dma_start(out=dbg["attn_d"][rs, :], in_=attn_bf[:, kt, :])
            for tt in range(TT):
                nc.gpsimd.dma_start(out=dbg["V_d"][128 * tt:128 * tt + 128, :],
                                  in_=V_sb[:, tt, :])

        # ---------- Phase 4: Wo + residual + LN2 ----------
        xTq_sb = sb.tile([128, KT, QL], F32, tag="fatA")
        nc.sync.dma_start(out=xTq_sb, in_=xTq.rearrange("(k p) s -> p k s", p=128))
        r_sb = sb.tile([128, KT, QL], F32, tag="fatB")
        for n in range(QL // 512):
            cs = slice(512 * n, 512 * n + 512)
            for mo in range(KT):
                wot = sb.tile([128, KT, 128], BF16, tag="wk6", bufs=3,
                              name=f"wo{n}_{mo}")
                nc.sync.dma_start(
                    out=wot,
                    in_=Wo.rearrange("(k p) m -> p k m", p=128)[:, :, 128 * mo:128 * mo + 128])
                pst = pst_tile(f"ops2{mo}_{n}", (128, 512))
                for kt in range(KT):
                    nc.tensor.matmul(pst, wot[:, kt, :], attn_bf[:, kt, cs],
                                     start=(kt == 0), stop=(kt == KT - 1))
                if biases_zero:
                    nc.vector.tensor_add(r_sb[:, mo, cs], pst, xTq_sb[:, mo, cs])
                else:
                    nc.vector.scalar_tensor_tensor(r_sb[:, mo, cs], pst,
                                                   par[:, 36 + mo:37 + mo],
                                                   xTq_sb[:, mo, cs],
                                                   Alu.add, Alu.add)
        if debug:
            for kt in range(KT):
                nc.sync.dma_start(out=dbg["r_d"][128 * kt:128 * kt + 128, :],
                                  in_=r_sb[:, kt, :])
        rb_sb = sb.tile([128, KT, QL], BF16, tag="medB")
        for kt in range(KT):
            nc.vector.tensor_copy(rb_sb[:, kt, :], r_sb[:, kt, :])
        ln2_bf = sb.tile([128, KT, QL], BF16, tag="medC")
        emit_ln(QL, rb_sb, ln2_bf, 8)

        if debug:
            for kt in range(KT):
                nc.gpsimd.dma_start(out=dbg["ln2_d"][128 * kt:128 * kt + 128, :],
                                  in_=ln2_bf[:, kt, :])
        # ---------- Phase 5: MLP (token-half sweep, g stays in SBUF) ----------
        y_sb = sb.tile([128, KT, QL], F32, tag="fatC")
        for n in range(QL // 512):
            cs = slice(512 * n, 512 * n + 512)
            g_half = sb.tile([128, MH, 512], BF16, tag="vsb", name=f"gh{n}")
            for mo in range(MH):
                w1t = sb.tile([128, KT, 128], BF16, tag="wk6", bufs=3,
                              name=f"w1{n}_{mo}")
                nc.sync.dma_start(
                    out=w1t,
                    in_=W1.rearrange("(k p) m -> p k m", p=128)[:, :, 128 * mo:128 * mo + 128])
                pst = pst_tile(f"h1ps{n}_{mo}", (128, 512))
                for kt in range(KT):
                    nc.tensor.matmul(pst, w1t[:, kt, :], ln2_bf[:, kt, cs],
                                     start=(kt == 0), stop=(kt == KT - 1))
                if biases_zero:
                    nc.scalar.activation(g_half[:, mo, :], pst, Act.Gelu)
                else:
                    nc.scalar.activation(g_half[:, mo, :], pst, Act.Gelu,
                                         bias=par[:, 48 + mo:49 + mo], scale=1.0)
            psts = [pst_tile(f"yps{n}_{i}") for i in range(3)]
            for kp in range(MH // 2):
                w2t = sb.tile([128, 2, HID], BF16, tag="w2", bufs=2, name=f"w2{n}_{kp}")
                nc.sync.dma_start(
                    out=w2t,
                    in_=W2.rearrange("(a p) m -> p a m", p=128)[:, 2 * kp:2 * kp + 2, :])
                for j in range(2):
                    k2 = 2 * kp + j
                    for mo in range(KT):
                        nc.tensor.matmul(
                            psts[mo // 2][:, 512 * (mo % 2):512 * (mo % 2) + 512],
                            w2t[:, j, 128 * mo:128 * mo + 128],
                            g_half[:, k2, :],
                            start=(k2 == 0), stop=(k2 == MH - 1))
            for mo in range(KT):
                pslice = psts[mo // 2][:, 512 * (mo % 2):512 * (mo % 2) + 512]
                if biases_zero:
                    nc.vector.tensor_add(y_sb[:, mo, cs], pslice, r_sb[:, mo, cs])
                else:
                    nc.vector.scalar_tensor_tensor(y_sb[:, mo, cs], pslice,
                                                   par[:, 42 + mo:43 + mo],
                                                   r_sb[:, mo, cs],
                                                   Alu.add, Alu.add)
                nc.sync.dma_start(out=y[128 * mo:128 * mo + 128, cs],
                                  in_=y_sb[:, mo, cs])

    nc.compile()
    return nc


def _get_program(biases_zero):
    key = ("nc", biases_zero)
    if key not in _CACHE:
        _CACHE[key] = _build_program(biases_zero)
    return _CACHE[key]


def _prep_in_maps(inputs):
    import ml_dtypes
    bf = ml_dtypes.bfloat16
    f32 = np.float32

    x = np.ascontiguousarray(np.asarray(inputs["x"], dtype=f32))
    ln1w = np.asarray(inputs["ln1_w"], f32)
    ln1b = np.asarray(inputs["ln1_b"], f32)
    ln2w = np.asarray(inputs["ln2_w"], f32)
    ln2b = np.asarray(inputs["ln2_b"], f32)
    Wq = np.asarray(inputs["Wq"], f32)
    Wk = np.asarray(inputs["Wk"], f32)
    Wv = np.asarray(inputs["Wv"], f32)
    Wo = np.asarray(inputs["Wo"], f32)
    W1 = np.asarray(inputs["W1"], f32)
    W2 = np.asarray(inputs["W2"], f32)
    # fold LN gains into consumer weights; LN bias contribution into proj biases
    Wq_f = ln1w[:, None] * Wq
    Wk_f = ln1w[:, None] * Wk
    Wv_f = ln1w[:, None] * Wv
    W1_f = ln2w[:, None] * W1
    bq_e = Wq.T @ ln1b + np.asarray(inputs["bq"], f32)
    bk_e = Wk.T @ ln1b + np.asarray(inputs["bk"], f32)
    bv_e = Wv.T @ ln1b + np.asarray(inputs["bv"], f32)
    b1_e = W1.T @ ln2b + np.asarray(inputs["b1"], f32)
    bo_e = np.asarray(inputs["bo"], f32)
    b2_e = np.asarray(inputs["b2"], f32)
    biases_zero = bool(
        all(np.all(v == 0) for v in (bq_e, bk_e, bv_e, b1_e, bo_e, b2_e)))

    shared = {
        "Wq": np.ascontiguousarray(Wq_f.astype(bf)),
        "Wk": np.ascontiguousarray(Wk_f.astype(bf)),
        "Wv": np.ascontiguousarray(Wv_f.astype(bf)),
        "Wo": np.ascontiguousarray(Wo.astype(bf)),
        "W1": np.ascontiguousarray(W1_f.astype(bf)),
        "W2": np.ascontiguousarray(W2.astype(bf)),
        "bqs": (bq_e / np.float32(np.sqrt(D))).astype(f32),
        "bk": bk_e, "bv": bv_e, "bo": bo_e, "b1": b1_e, "b2": b2_e,
    }

    in_maps = []
    qcols_all = []
    for c in range(N_CORES):
        b, p = c // 2, c % 2
        xT = np.ascontiguousarray(x[b].T)
        qcols = np.concatenate(
            [np.arange(128 * (2 * j + p), 128 * (2 * j + p) + 128) for j in range(8)])
        qcols_all.append(qcols)
        xTq = np.ascontiguousarray(xT[:, qcols])
        m = np.zeros((TT, 128, 128), np.float32)
        for t in range(TT):
            g = 2 * (t // 2) + p
            kk = 128 * t + np.arange(128)[:, None]
            qq = 128 * g + np.arange(128)[None, :]
            m[t] = (kk <= qq).astype(np.float32)
        im = dict(shared)
        im["xbT"] = xT.astype(bf)
        im["xbTq"] = xTq.astype(bf)
        im["xTq"] = xTq
        im["masks"] = m.astype(bf)
        in_maps.append(im)
    return in_maps, qcols_all, biases_zero


def kernel(**inputs):
    import sys, types
    if "antenv.axon_hooks" not in sys.modules:
        try:
            sys.path.insert(0, "/root/.axon_site")
            from trn_agent_boot.trn_boot import _ntff_profile_via_ctypes
            hook = _ntff_profile_via_ctypes("/opt/axon/libaxon_pjrt.so")
            mod = types.ModuleType("antenv.axon_hooks")
            mod.get_axon_ntff_profile_hook = lambda: hook
            mod.set_axon_ntff_profile_hook = lambda h: None
            import antenv  # noqa: F401
            sys.modules["antenv.axon_hooks"] = mod
        except Exception:
            pass

    from concourse.bass_utils import run_bass_kernel_spmd

    in_maps, qcols_all, biases_zero = _prep_in_maps(inputs)
    nc = _get_program(biases_zero)
    res = run_bass_kernel_spmd(nc, in_maps, core_ids=list(range(N_CORES)))
    out = np.zeros((B, S, HID), np.float32)
    for c in range(N_CORES):
        out[c // 2, qcols_all[c], :] = res.results[c]["y"].T
    return out


# revision 18
# speedup vs baseline: 1.4018x; 1.4018x over previous
"""Trainium2 Bass kernel for a pre-LN transformer block (B=4, S=2048, H=12, D=64).

Sharding: 8 cores; core c -> batch b = c//2, parity p = c%2.
Each core handles 1024 query rows of its batch: local query block j (128 rows,
j=0..7) maps to global block g = 2j + p (stride-2 interleave balances the causal
load so every core runs an identical SPMD program).

Per core (activations feature-major [feature, token]):
  LN1 (stats via ones-matmul over partitions, a/c bounced through DRAM for a
  [128,n] reshape, then broadcast), QKV projections, causal attention
  (scores transposed [keys, q], key-tile-outer; head pairs run concurrently on
  PE row-groups; softmax without max-subtraction; multiplicative masks from
  host; normalization decoupled from the accumulation critical path),
  Wo + residual, LN2 (fp32 stats), MLP (exact GELU, token-half sweeps, hidden
  activations stay in SBUF) + residual.

LayerNorm gains are folded into the consumer weights on the host
(W <- diag(ln_w) @ W), so the on-device LN is just u = x*rstd - mu*rstd.
When all bias vectors are zero (they are for this model), PSUM evacuations run
on the Scalar engine; otherwise a Vector-engine bias path is emitted.
"""

import numpy as np

N_CORES = 8
B, S, H, D = 4, 2048, 12, 64
HID = 768
QL = 1024
KT = HID // 128
TT = S // 128
MH = 4 * HID // 128
EPS = 1e-5

_CACHE = {}


def _build_program(biases_zero, debug=False):
    from contextlib import ExitStack
    import concourse.bass as bass
    import concourse.tile as tile
    from concourse import bacc, mybir

    F32 = mybir.dt.float32
    BF16 = mybir.dt.bfloat16
    Alu = mybir.AluOpType
    Act = mybir.ActivationFunctionType

    nc = bacc.Bacc("TRN2", target_bir_lowering=False, debug=False,
                   enable_asserts=False, num_devices=N_CORES)

    def din(name, shape, dt):
        return nc.dram_tensor(name, shape, dt, kind="ExternalInput").ap()

    xbT = din("xbT", [HID, S], BF16)
    xbTq = din("xbTq", [HID, QL], BF16)
    xTq = din("xTq", [HID, QL], F32)
    masks = din("masks", [TT, 128, 128], BF16)
    Wq = din("Wq", [HID, HID], BF16)      # pre-folded with ln1_w on host
    Wk = din("Wk", [HID, HID], BF16)
    Wv = din("Wv", [HID, HID], BF16)
    Wo = din("Wo", [HID, HID], BF16)
    W1 = din("W1", [HID, 4 * HID], BF16)  # pre-folded with ln2_w
    W2 = din("W2", [4 * HID, HID], BF16)
    bqs = din("bqs", [HID], F32)          # (Wq^T ln1_b + bq)/sqrt(D)
    bk = din("bk", [HID], F32)            # Wk^T ln1_b + bk
    bv = din("bv", [HID], F32)
    bo = din("bo", [HID], F32)
    b1 = din("b1", [4 * HID], F32)        # W1^T ln2_b + b1
    b2 = din("b2", [HID], F32)

    y = nc.dram_tensor("y", [HID, QL], F32, kind="ExternalOutput").ap()
    dbg = {}
    if debug:
        for nm, shp in [("ln_d", [HID, S]), ("lnq_d", [HID, QL]),
                        ("K_d", [HID, S]), ("Q_d", [HID, QL]),
                        ("V_d", [TT * 128, H * 65]), ("attn_d", [HID, QL]),
                        ("r_d", [HID, QL]), ("ln2_d", [HID, QL])]:
            dbg[nm] = nc.dram_tensor(nm, shp, F32, kind="ExternalOutput").ap()

    st_dram = nc.dram_tensor("st_scratch", [16, S], F32).ap()
    rec_dram = nc.dram_tensor("rec_scratch", [H, QL], F32).ap()
    rec2_dram = nc.dram_tensor("rec2_scratch", [H, QL], F32).ap()

    def bcast(src_elem_ap, parts, n):
        return bass.AP(tensor=src_elem_ap.tensor, offset=src_elem_ap.offset,
                       ap=[[0, parts], [1, n]])

    def resh_ap(dram, row_elem_ap, nb):
        return bass.AP(tensor=dram.tensor, offset=row_elem_ap.offset,
                       ap=[[1, 128], [128, nb]])

    with tile.TileContext(nc) as tc, ExitStack() as ctx:
        sb = ctx.enter_context(tc.tile_pool(name="sb", bufs=1))
        ps = ctx.enter_context(tc.tile_pool(name="ps", bufs=1, space="PSUM"))

        def pst_tile(name, shape=(128, 1024), dt=F32):
            return ps.tile(list(shape), dt, tag="s2", bufs=4, name=name,
                           padded_shape=[128, 1024])

        # ---------- constants ----------
        ones_bf = sb.tile([128, 1], BF16, tag="ones")
        nc.vector.memset(ones_bf, 1.0)
        ones_f = sb.tile([128, 1], F32, tag="onesf")
        nc.vector.memset(ones_f, 1.0)
        par = sb.tile([128, 80], F32, tag="par")

        def load_cols(dst0, src, n):
            nc.gpsimd.dma_start(
                out=par[:, dst0:dst0 + n],
                in_=bass.AP(tensor=src.tensor, offset=src.offset,
                            ap=[[1, 128], [128, n]]))

        # cols: 24:30 bqs, 30:36 bk, 36:42 bo, 42:48 b2, 48:72 b1, 72 eps
        load_cols(24, bqs, KT)
        load_cols(30, bk, KT)
        load_cols(36, bo, KT)
        load_cols(42, b2, KT)
        load_cols(48, b1, MH)
        nc.vector.memset(par[:, 72:73], EPS)
        eps_t = par[:, 72:73]
        bv_b = sb.tile([128, HID], F32, tag="bv_b")
        nc.gpsimd.dma_start(out=bv_b, in_=bcast(bv[0], 128, HID))

        # PE warmup: dummy matmuls with no data deps lift the HAM clock gate
        warm = sb.tile([128, 512], BF16, tag="sq", bufs=2, name="warm")
        nc.vector.memset(warm, 0.0)
        wps = pst_tile("warmps", (1, 512))
        for i in range(20):
            nc.tensor.matmul(wps, ones_bf, warm, start=True, stop=True)

        masks_sb = sb.tile([128, TT, 128], BF16, tag="masks")
        nc.gpsimd.dma_start(out=masks_sb, in_=masks.rearrange("t p q -> p t q"))

        # ---------- LN helper: out = x*rstd - mu*rstd (gains folded into W) ----
        def emit_ln(N, x_bf, out_bf, row0, f32_stats=False):
            nchunks = N // 512
            one = ones_f if f32_stats else ones_bf
            for c in range(nchunks):
                off = 512 * c
                cs = slice(off, off + 512)
                s_ps = pst_tile(f"sps{row0}_{c}", (1, 512))
                q_ps = pst_tile(f"qps{row0}_{c}", (1, 512))
                for kt in range(KT):
                    nc.tensor.matmul(s_ps, one, x_bf[:, kt, cs],
                                     start=(kt == 0), stop=(kt == KT - 1))
                for kt in range(KT):
                    sqd = F32 if f32_stats else BF16
                    sqc = sb.tile([128, 512], sqd, tag="sq", bufs=2,
                                  name=f"sq{row0}_{c}_{kt}")
                    nc.vector.tensor_mul(sqc, x_bf[:, kt, cs], x_bf[:, kt, cs])
                    nc.tensor.matmul(q_ps, one, sqc,
                                     start=(kt == 0), stop=(kt == KT - 1))
                s_sb = sb.tile([1, 512], F32, tag="sts", bufs=2, name=f"ssb{row0}_{c}")
                q_sb = sb.tile([1, 512], F32, tag="sts", bufs=2, name=f"qsb{row0}_{c}")
                nc.scalar.copy(s_sb, s_ps)
                nc.scalar.copy(q_sb, q_ps)
                nc.gpsimd.dma_start(out=st_dram[row0, off:off + 512], in_=s_sb)
                nc.gpsimd.dma_start(out=st_dram[row0 + 1, off:off + 512], in_=q_sb)
                s_r = sb.tile([128, 4], F32, tag="str", bufs=8, name=f"sr{row0}_{c}")
                q_r = sb.tile([128, 4], F32, tag="str", bufs=8, name=f"qr{row0}_{c}")
                nc.gpsimd.dma_start(out=s_r, in_=resh_ap(st_dram, st_dram[row0, off], 4))
                nc.gpsimd.dma_start(out=q_r, in_=resh_ap(st_dram, st_dram[row0 + 1, off], 4))
                mu = sb.tile([128, 4], F32, tag="str", bufs=8, name=f"mu{row0}_{c}")
                e2 = sb.tile([128, 4], F32, tag="str", bufs=8, name=f"e2{row0}_{c}")
                nc.scalar.mul(mu, s_r, 1.0 / HID)
                nc.scalar.mul(e2, q_r, 1.0 / HID)
                var = sb.tile([128, 4], F32, tag="str", bufs=8, name=f"var{row0}_{c}")
                nc.vector.tensor_mul(var, mu, mu)
                nc.vector.tensor_sub(var, e2, var)
                sd = sb.tile([128, 4], F32, tag="str", bufs=8, name=f"sd{row0}_{c}")
                nc.scalar.activation(sd, var, Act.Sqrt, bias=eps_t, scale=1.0)
                a_r = sb.tile([128, 4], F32, tag="str", bufs=8, name=f"ar{row0}_{c}")
                nc.vector.reciprocal(a_r, sd)
                c_r = sb.tile([128, 4], F32, tag="str", bufs=8, name=f"cr{row0}_{c}")
                nc.vector.tensor_mul(c_r, mu, a_r)
                nc.scalar.mul(c_r, c_r, -1.0)
                nc.gpsimd.dma_start(out=resh_ap(st_dram, st_dram[row0 + 2, off], 4), in_=a_r)
                nc.gpsimd.dma_start(out=resh_ap(st_dram, st_dram[row0 + 3, off], 4), in_=c_r)
                a_b = sb.tile([128, 512], F32, tag="ab", bufs=2, name=f"ab{row0}_{c}")
                c_b = sb.tile([128, 512], F32, tag="ab", bufs=2, name=f"cb{row0}_{c}")
                nc.gpsimd.dma_start(out=a_b, in_=bcast(st_dram[row0 + 2, off], 128, 512))
                nc.gpsimd.dma_start(out=c_b, in_=bcast(st_dram[row0 + 3, off], 128, 512))
                for kt in range(KT):
                    t0 = sb.tile([128, 512], F32, tag="t0", bufs=2,
                                 name=f"t0{row0}_{c}_{kt}")
                    nc.vector.tensor_mul(t0, x_bf[:, kt, cs], a_b)
                    nc.vector.tensor_add(out_bf[:, kt, cs], t0, c_b)

        # evacuation helper: psum -> sbuf bf16 (+ optional bias col, scale)
        def evac(dst, src, bias_col=None, scale=1.0):
            if biases_zero or bias_col is None:
                if scale == 1.0:
                    nc.scalar.copy(dst, src)
                else:
                    nc.scalar.mul(dst, src, scale)
            else:
                nc.vector.tensor_scalar(dst, src, scale, bias_col,
                                        Alu.mult, Alu.add)

        # ---------- Phase 1: LN1 + LN1q ----------
        xbT_sb = sb.tile([128, KT, S], BF16, tag="fatA")
        for c in range(S // 512):
            nc.sync.dma_start(
                out=xbT_sb[:, :, 512 * c:512 * c + 512],
                in_=xbT.rearrange("(k p) s -> p k s", p=128)[:, :, 512 * c:512 * c + 512])
        xbTq_sb = sb.tile([128, KT, QL], BF16, tag="medB")
        for c in range(QL // 512):
            nc.sync.dma_start(
                out=xbTq_sb[:, :, 512 * c:512 * c + 512],
                in_=xbTq.rearrange("(k p) s -> p k s", p=128)[:, :, 512 * c:512 * c + 512])

        ln_bf = sb.tile([128, KT, S], BF16, tag="fatB")
        lnq_bf = sb.tile([128, KT, QL], BF16, tag="medA")
        emit_ln(S, xbT_sb, ln_bf, 0)
        emit_ln(QL, xbTq_sb, lnq_bf, 4)

        # ---------- Phase 2: QKV ----------
        K_sb = sb.tile([128, KT, S], BF16, tag="fatC")
        for n in range(S // 512):
            cs = slice(512 * n, 512 * n + 512)
            for mo in range(KT):
                wkt = sb.tile([128, KT, 128], BF16, tag="wk6", bufs=3,
                              name=f"wk{n}_{mo}")
                nc.sync.dma_start(
                    out=wkt,
                    in_=Wk.rearrange("(k p) m -> p k m", p=128)[:, :, 128 * mo:128 * mo + 128])
                pst = pst_tile(f"kps{mo}_{n}", (128, 512))
                for kt in range(KT):
                    nc.tensor.matmul(pst, wkt[:, kt, :], ln_bf[:, kt, cs],
                                     start=(kt == 0), stop=(kt == KT - 1))
                evac(K_sb[:, mo, cs], pst, par[:, 30 + mo:31 + mo])

        Q_sb = sb.tile([128, KT, QL], BF16, tag="qsb")
        for n in range(QL // 512):
            cs = slice(512 * n, 512 * n + 512)
            for mo in range(KT):
                wqt = sb.tile([128, KT, 128], BF16, tag="wk6", bufs=3,
                              name=f"wq{n}_{mo}")
                nc.sync.dma_start(
                    out=wqt,
                    in_=Wq.rearrange("(k p) m -> p k m", p=128)[:, :, 128 * mo:128 * mo + 128])
                pst = pst_tile(f"qps2{mo}_{n}", (128, 512))
                for kt in range(KT):
                    nc.tensor.matmul(pst, wqt[:, kt, :], lnq_bf[:, kt, cs],
                                     start=(kt == 0), stop=(kt == KT - 1))
                evac(Q_sb[:, mo, cs], pst, par[:, 24 + mo:25 + mo], scale=0.125)

        V_sb = sb.tile([128, TT, H * 65], BF16, tag="vsb")
        for fc in range(2):
            wvt = sb.tile([128, KT, 384], BF16, tag="wv", bufs=1, name=f"wv{fc}")
            nc.sync.dma_start(
                out=wvt,
                in_=Wv.rearrange("(k p) m -> p k m", p=128)[:, :, 384 * fc:384 * fc + 384])
            for tt in range(TT):
                pst = pst_tile(f"vps{tt}_{fc}", (128, 384))
                for kt in range(KT):
                    nc.tensor.matmul(pst, ln_bf[:, kt, 128 * tt:128 * tt + 128],
                                     wvt[:, kt, :],
                                     start=(kt == 0), stop=(kt == KT - 1))
                vdst = V_sb[:, tt, :].rearrange("p (h e) -> p h e", e=65)[:, 6 * fc:6 * fc + 6, 0:64]
                if biases_zero:
                    nc.scalar.copy(vdst, pst.rearrange("p (h d) -> p h d", d=64))
                else:
                    nc.vector.tensor_tensor(
                        vdst, pst.rearrange("p (h d) -> p h d", d=64),
                        bv_b[:, 384 * fc:384 * fc + 384].rearrange("p (h d) -> p h d", d=64),
                        Alu.add)
        for tt in range(TT):
            nc.vector.memset(
                V_sb[:, tt, :].rearrange("p (h e) -> p h e", e=65)[:, :, 64:65], 1.0)

        # ---------- Phase 3: attention (head pairs on PE row groups) ----------
        attn_bf = sb.tile([128, KT, QL], BF16, tag="medA")
        for kt in range(KT):
            h0, h1 = 2 * kt, 2 * kt + 1
            O = {h0: pst_tile(f"o{h0}", (65, QL)),
                 h1: pst_tile(f"o{h1}", (65, QL))}

            def av(h, tile_t, e, q0):
                for (cs, ce) in ([(q0, 512), (512, QL)] if q0 < 512 else [(q0, QL)]):
                    nc.tensor.matmul(O[h][:, cs:ce],
                                     V_sb[:, tile_t, 65 * h:65 * h + 65],
                                     e[:, cs - q0:ce - q0],
                                     start=(tile_t == 0), stop=(tile_t == TT - 1))

            prev = []
            for t in range(TT):
                q0 = 128 * (t // 2)
                span = QL - q0
                cur = []
                for h, pr in ((h0, slice(0, 64)), (h1, slice(64, 128))):
                    S_ps = pst_tile(f"scr{h}_{t}", (128, QL))
                    for (cs, ce) in ([(q0, 512), (512, QL)] if q0 < 512
                                     else [(q0, QL)]):
                        nc.tensor.matmul(S_ps[:, cs:ce],
                                         K_sb[pr, kt, 128 * t:128 * t + 128],
                                         Q_sb[pr, kt, cs:ce], start=True, stop=True)
                    expS = sb.tile([128, span], BF16, tag="expS", bufs=4,
                                   name=f"es{h}_{t}")
                    nc.scalar.activation(expS, S_ps[:, q0:QL], Act.Exp)
                    nc.vector.tensor_mul(expS[:, 0:128], expS[:, 0:128],
                                         masks_sb[:, t, :])
                    cur.append((h, expS, q0, t))
                for (h, e, pq0, pt) in prev:
                    av(h, pt, e, pq0)
                prev = cur
            for (h, e, pq0, pt) in prev:
                av(h, pt, e, pq0)
            # fast raw evacuation (frees O); normalization decoupled below
            for h in (h0, h1):
                rec = sb.tile([65, QL], F32, tag="rec", bufs=2, name=f"rec{h}")
                nc.vector.tensor_copy(rec[64:65, :], O[h][64:65, :])
                if h % 2 == 0:
                    nc.vector.tensor_copy(attn_bf[0:64, kt, :], O[h][0:64, :])
                else:
                    stg = sb.tile([64, QL], BF16, tag="stg", bufs=1, name=f"stg{h}")
                    nc.vector.tensor_copy(stg, O[h][0:64, :])
                nc.gpsimd.dma_start(out=rec_dram[h, :], in_=rec[64:65, :])
                rr = sb.tile([128, 8], F32, tag="str", bufs=8, name=f"rr{h}")
                nc.gpsimd.dma_start(out=rr, in_=resh_ap(rec_dram, rec_dram[h, 0], 8))
                rr2 = sb.tile([128, 8], F32, tag="str", bufs=8, name=f"rr2{h}")
                nc.vector.reciprocal(rr2, rr)
                nc.gpsimd.dma_start(out=resh_ap(rec2_dram, rec2_dram[h, 0], 8), in_=rr2)
                nc.gpsimd.dma_start(out=rec[0:64, :], in_=bcast(rec2_dram[h, 0], 64, QL))
                if h % 2 == 0:
                    nc.vector.tensor_mul(attn_bf[0:64, kt, :],
                                         attn_bf[0:64, kt, :], rec[0:64, :])
                else:
                    nc.vector.tensor_mul(stg, stg, rec[0:64, :])
                    nc.sync.dma_start(out=attn_bf[64:128, kt, :], in_=stg)

        if debug:
            for kt in range(KT):
                rs = slice(128 * kt, 128 * kt + 128)
                nc.gpsimd.dma_start(out=dbg["ln_d"][rs, :], in_=ln_bf[:, kt, :])
                nc.gpsimd.dma_start(out=dbg["lnq_d"][rs, :], in_=lnq_bf[:, kt, :])
                nc.gpsimd.dma_start(out=dbg["K_d"][rs, :], in_=K_sb[:, kt, :])
                nc.gpsimd.dma_start(out=dbg["Q_d"][rs, :], in_=Q_sb[:, kt, :])
                nc.gpsimd.dma_start(out=dbg["attn_d"][rs, :], in_=attn_bf[:, kt, :])
            for tt in range(TT):
                nc.gpsimd.dma_start(out=dbg["V_d"][128 * tt:128 * tt + 128, :],
                                  in_=V_sb[:, tt, :])

        # ---------- Phase 4: Wo + residual + LN2 ----------
        xTq_sb = sb.tile([128, KT, QL], F32, tag="fatA")
        nc.sync.dma_start(out=xTq_sb, in_=xTq.rearrange("(k p) s -> p k s", p=128))
        r_sb = sb.tile([128, KT, QL], F32, tag="fatB")
        for n in range(QL // 512):
            cs = slice(512 * n, 512 * n + 512)
            for mo in range(KT):
                wot = sb.tile([128, KT, 128], BF16, tag="wk6", bufs=3,
                              name=f"wo{n}_{mo}")
                nc.sync.dma_start(
                    out=wot,
                    in_=Wo.rearrange("(k p) m -> p k m", p=128)[:, :, 128 * mo:128 * mo + 128])
                pst = pst_tile(f"ops2{mo}_{n}", (128, 512))
                for kt in range(KT):
                    nc.tensor.matmul(pst, wot[:, kt, :], attn_bf[:, kt, cs],
                                     start=(kt == 0), stop=(kt == KT - 1))
                if biases_zero:
                    nc.vector.tensor_add(r_sb[:, mo, cs], pst, xTq_sb[:, mo, cs])
                else:
                    nc.vector.scalar_tensor_tensor(r_sb[:, mo, cs], pst,
                                                   par[:, 36 + mo:37 + mo],
                                                   xTq_sb[:, mo, cs],
                                                   Alu.add, Alu.add)
        if debug:
            for kt in range(KT):
                nc.sync.dma_start(out=dbg["r_d"][128 * kt:128 * kt + 128, :],
                                  in_=r_sb[:, kt, :])
        rb_sb = sb.tile([128, KT, QL], BF16, tag="medB")
        for kt in range(KT):
            nc.vector.tensor_copy(rb_sb[:, kt, :], r_sb[:, kt, :])
        ln2_bf = sb.tile([128, KT, QL], BF16, tag="medC")
        emit_ln(QL, rb_sb, ln2_bf, 8)

        if debug:
            for kt in range(KT):
                nc.gpsimd.dma_start(out=dbg["ln2_d"][128 * kt:128 * kt + 128, :],
                                  in_=ln2_bf[:, kt, :])
        # ---------- Phase 5: MLP (token-half sweep, g stays in SBUF) ----------
        y_sb = sb.tile([128, KT, QL], F32, tag="fatC")
        for n in range(QL // 512):
            cs = slice(512 * n, 512 * n + 512)
            g_half = sb.tile([128, MH, 512], BF16, tag="vsb", name=f"gh{n}")
            for mo in range(MH):
                w1t = sb.tile([128, KT, 128], BF16, tag="wk6", bufs=3,
                              name=f"w1{n}_{mo}")
                nc.sync.dma_start(
                    out=w1t,
                    in_=W1.rearrange("(k p) m -> p k m", p=128)[:, :, 128 * mo:128 * mo + 128])
                pst = pst_tile(f"h1ps{n}_{mo}", (128, 512))
                for kt in range(KT):
                    nc.tensor.matmul(pst, w1t[:, kt, :], ln2_bf[:, kt, cs],
                                     start=(kt == 0), stop=(kt == KT - 1))
                if biases_zero:
                    nc.scalar.activation(g_half[:, mo, :], pst, Act.Gelu)
                else:
                    nc.scalar.activation(g_half[:, mo, :], pst, Act.Gelu,
                                         bias=par[:, 48 + mo:49 + mo], scale=1.0)
            psts = [pst_tile(f"yps{n}_{i}") for i in range(3)]
            for kp in range(MH // 2):
                w2t = sb.tile([128, 2, HID], BF16, tag="w2", bufs=2, name=f"w2{n}_{kp}")
                nc.sync.dma_start(
                    out=w2t,
                    in_=W2.rearrange("(a p) m -> p a m", p=128)[:, 2 * kp:2 * kp + 2, :])
                for j in range(2):
                    k2 = 2 * kp + j
                    for mo in range(KT):
                        nc.tensor.matmul(
                            psts[mo // 2][:, 512 * (mo % 2):512 * (mo % 2) + 512],
                            w2t[:, j, 128 * mo:128 * mo + 128],
                            g_half[:, k2, :],
                            start=(k2 == 0), stop=(k2 == MH - 1))
            for mo in range(KT):
                pslice = psts[mo // 2][:, 512 * (mo % 2):512 * (mo % 2) + 512]
                if biases_zero:
                    nc.vector.tensor_add(y_sb[:, mo, cs], pslice, r_sb[:, mo, cs])
                else:
                    nc.vector.scalar_tensor_tensor(y_sb[:, mo, cs], pslice,
                                                   par[:, 42 + mo:43 + mo],
                                                   r_sb[:, mo, cs],
                                                   Alu.add, Alu.add)
                nc.sync.dma_start(out=y[128 * mo:128 * mo + 128, cs],
                                  in_=y_sb[:, mo, cs])

    nc.compile()
    return nc


def _get_program(biases_zero):
    key = ("nc", biases_zero)
    if key not in _CACHE:
        _CACHE[key] = _build_program(biases_zero)
    return _CACHE[key]


def _prep_in_maps(inputs):
    import ml_dtypes
    bf = ml_dtypes.bfloat16
    f32 = np.float32

    x = np.ascontiguousarray(np.asarray(inputs["x"], dtype=f32))
    ln1w = np.asarray(inputs["ln1_w"], f32)
    ln1b = np.asarray(inputs["ln1_b"], f32)
    ln2w = np.asarray(inputs["ln2_w"], f32)
    ln2b = np.asarray(inputs["ln2_b"], f32)
    Wq = np.asarray(inputs["Wq"], f32)
    Wk = np.asarray(inputs["Wk"], f32)
    Wv = np.asarray(inputs["Wv"], f32)
    Wo = np.asarray(inputs["Wo"], f32)
    W1 = np.asarray(inputs["W1"], f32)
    W2 = np.asarray(inputs["W2"], f32)
    # fold LN gains into consumer weights; LN bias contribution into proj biases
    Wq_f = ln1w[:, None] * Wq
    Wk_f = ln1w[:, None] * Wk
    Wv_f = ln1w[:, None] * Wv
    W1_f = ln2w[:, None] * W1
    bq_e = Wq.T @ ln1b + np.asarray(inputs["bq"], f32)
    bk_e = Wk.T @ ln1b + np.asarray(inputs["bk"], f32)
    bv_e = Wv.T @ ln1b + np.asarray(inputs["bv"], f32)
    b1_e = W1.T @ ln2b + np.asarray(inputs["b1"], f32)
    bo_e = np.asarray(inputs["bo"], f32)
    b2_e = np.asarray(inputs["b2"], f32)
    biases_zero = bool(
        all(np.all(v == 0) for v in (bq_e, bk_e, bv_e, b1_e, bo_e, b2_e)))

    shared = {
        "Wq": np.ascontiguousarray(Wq_f.astype(bf)),
        "Wk": np.ascontiguousarray(Wk_f.astype(bf)),
        "Wv": np.ascontiguousarray(Wv_f.astype(bf)),
        "Wo": np.ascontiguousarray(Wo.astype(bf)),
        "W1": np.ascontiguousarray(W1_f.astype(bf)),
        "W2": np.ascontiguousarray(W2.astype(bf)),
        "bqs": (bq_e / np.float32(np.sqrt(D))).astype(f32),
        "bk": bk_e, "bv": bv_e, "bo": bo_e, "b1": b1_e, "b2": b2_e,
    }

    in_maps = []
    qcols_all = []
    for c in range(N_CORES):
        b, p = c // 2, c % 2
        xT = np.ascontiguousarray(x[b].T)
        qcols = np.concatenate(
            [np.arange(128 * (2 * j + p), 128 * (2 * j + p) + 128) for j in range(8)])
        qcols_all.append(qcols)
        xTq = np.ascontiguousarray(xT[:, qcols])
        m = np.zeros((TT, 128, 128), np.float32)
        for t in range(TT):
            g = 2 * (t // 2) + p
            kk = 128 * t + np.arange(128)[:, None]
            qq = 128 * g + np.arange(128)[None, :]
            m[t] = (kk <= qq).astype(np.float32)
        im = dict(shared)
        im["xbT"] = xT.astype(bf)
        im["xbTq"] = xTq.astype(bf)
        im["xTq"] = xTq
        im["masks"] = m.astype(bf)
        in_maps.append(im)
    return in_maps, qcols_all, biases_zero


def kernel(**inputs):
    import sys, types
    if "antenv.axon_hooks" not in sys.modules:
        try:
            sys.path.insert(0, "/root/.axon_site")
            from trn_agent_boot.trn_boot import _ntff_profile_via_ctypes
            hook = _ntff_profile_via_ctypes("/opt/axon/libaxon_pjrt.so")
            mod = types.ModuleType("antenv.axon_hooks")
            mod.get_axon_ntff_profile_hook = lambda: hook
            mod.set_axon_ntff_profile_hook = lambda h: None
            import antenv  # noqa: F401
            sys.modules["antenv.axon_hooks"] = mod
        except Exception:
            pass

    from concourse.bass_utils import run_bass_kernel_spmd

    in_maps, qcols_all, biases_zero = _prep_in_maps(inputs)
    nc = _get_program(biases_zero)
    res = run_bass_kernel_spmd(nc, in_maps, core_ids=list(range(N_CORES)))
    out = np.zeros((B, S, HID), np.float32)
    for c in range(N_CORES):
        out[c // 2, qcols_all[c], :] = res.results[c]["y"].T
    return out
